# revision 1
# baseline (speedup 1.0000x reference)
"""Self-contained Trainium2 Bass kernel for sparse attention.

Sharding: 8 cores = (image b, L-half). Each core receives its image's x0
ROTATED so its own 4096 rows come first (gather indices are remapped on
the host to match). The core computes LN+K/V for all 8192 rows, writes
packed bf16 [k|v] rows to DRAM scratch, then per 128-row tile gathers
2048 neighbor rows with dma_gather and runs attention + merge + MLP +
LN2 fully on-chip. No collectives.
"""
import numpy as np
import ml_dtypes

import concourse.bass as bass
import concourse.tile as tile
from concourse import bacc, mybir

F32 = mybir.dt.float32
BF16 = mybir.dt.bfloat16
I16 = mybir.dt.int16
I32 = mybir.dt.int32
AX = mybir.AxisListType
OP = mybir.AluOpType
AF = mybir.ActivationFunctionType
ts = bass.ts

L, C, NJ, NH, HD = 8192, 128, 16, 8, 16
LH = L // 2            # rows computed per core
NT_FULL = L // 128     # 64 k/v tiles
NT_HALF = LH // 128    # 32 attention tiles
EPS = 1e-5


def build_nc(nontrivial_ln1: bool, nontrivial_ln2: bool):
    nc = bacc.Bacc(None, target_bir_lowering=False, debug=False)

    x0f = nc.declare_dram_parameter("x0f", [L, C], F32, isOutput=False)
    gidx = nc.declare_dram_parameter("gidx", [128, NT_HALF * NJ], I32, isOutput=False)
    wnames = ["wq", "wk", "wv", "wm", "w1", "w2"]
    wparams = {n: nc.declare_dram_parameter(n, [C, C], BF16, isOutput=False) for n in wnames}
    identp = nc.declare_dram_parameter("ident", [C, C], BF16, isOutput=False)
    if nontrivial_ln1:
        bqkv = nc.declare_dram_parameter("bqkv", [1, 3 * C], F32, isOutput=False)
    if nontrivial_ln2:
        g2b2 = nc.declare_dram_parameter("g2b2", [1, 2 * C], F32, isOutput=False)
    out = nc.declare_dram_parameter("out", [LH, C], F32, isOutput=True)

    with tile.TileContext(nc) as tc:
        with (
            tc.tile_pool(name="res", bufs=1) as res,
            tc.tile_pool(name="dram", bufs=1, space="DRAM") as dram,
        ):
            kv_dram = dram.tile([L, 2 * C], BF16)
            x0_res = res.tile([128, NT_HALF * 128], F32)   # our-half x0 tiles
            q_res = res.tile([128, NT_HALF * 128], BF16)   # our-half q tiles
            idx_res = res.tile([128, NT_HALF * NJ], I32)
            ident = res.tile([128, 128], BF16)
            wsb = {n: res.tile([C, C], BF16, name=f"w_{n}", tag=f"w_{n}") for n in wnames}

            nc.sync.dma_start(idx_res[:], gidx[:])
            for n in wnames:
                nc.sync.dma_start(wsb[n][:], wparams[n][:])
            nc.sync.dma_start(ident[:], identp[:])
            if nontrivial_ln1:
                bqkv_sb = res.tile([1, 3 * C], F32)
                nc.sync.dma_start(bqkv_sb[:], bqkv[:])
            if nontrivial_ln2:
                g2b2_sb = res.tile([1, 2 * C], F32)
                nc.sync.dma_start(g2b2_sb[:], g2b2[:])

            # ---------------- Phase 1: LN1 + K/V (+Q) projections ----------------
            with (
                tc.tile_pool(name="p1", bufs=3) as p1,
                tc.tile_pool(name="p1s", bufs=2) as p1s,
                tc.tile_pool(name="ps1", bufs=2, space="PSUM") as ps1,
            ):
                for t in range(NT_FULL):
                    ours = t < NT_HALF
                    if ours:
                        x0t = x0_res[:, ts(t, 128)]
                    else:
                        x0t_tile = p1.tile([128, 128], F32, tag="x0t")
                        x0t = x0t_tile[:]
                    nc.sync.dma_start(x0t, x0f[ts(t, 128), :])

                    # LN1 stats
                    ssum = p1s.tile([128, 1], F32, tag="ssum")
                    mu = p1s.tile([128, 1], F32, tag="mu")
                    sq = p1s.tile([128, 128], F32, tag="sq")
                    m2 = p1s.tile([128, 1], F32, tag="m2")
                    var = p1s.tile([128, 1], F32, tag="var")
                    std = p1s.tile([128, 1], F32, tag="std")
                    rstd = p1s.tile([128, 1], F32, tag="rstd")
                    xn = p1.tile([128, 128], BF16, tag="xn")
                    nc.scalar.activation(sq[:], x0t, AF.Square, accum_out=m2[:])
                    nc.scalar.activation(sqc := p1s.tile([128, 128], BF16, tag="sqc", name="sqc"),
                                         x0t, AF.Copy, accum_out=ssum[:])
                    nc.vector.tensor_scalar_mul(mu[:], ssum[:], 1.0 / C)
                    nc.vector.tensor_scalar(
                        var[:], mu[:], scalar1=mu[:], scalar2=None, op0=OP.mult
                    )
                    nc.vector.tensor_scalar(
                        m2s2 := p1s.tile([128, 1], F32, tag="m2s2", name="m2s2"),
                        m2[:], scalar1=1.0 / C, scalar2=EPS, op0=OP.mult, op1=OP.add,
                    )
                    nc.vector.tensor_tensor(var[:], m2s2, var[:], op=OP.subtract)
                    nc.scalar.activation(std[:], var[:], AF.Sqrt)
                    nc.vector.reciprocal(rstd[:], std[:])
                    nc.vector.tensor_scalar(
                        xn[:], x0t, scalar1=mu[:], scalar2=rstd[:],
                        op0=OP.subtract, op1=OP.mult,
                    )

                    # transpose xn -> xnT (bf16)
                    xnT_ps = ps1.tile([128, 128], BF16, tag="xnT_ps")
                    xnT = p1.tile([128, 128], BF16, tag="xnT")
                    nc.tensor.transpose(xnT_ps[:], xn[:], ident[:])
                    nc.scalar.copy(xnT[:], xnT_ps[:])

                    # k/v projections -> packed [k|v] bf16 rows
                    kvt = p1.tile([128, 2 * C], BF16, tag="kvt")
                    k_ps = ps1.tile([128, 128], F32, tag="k_ps")
                    v_ps = ps1.tile([128, 128], F32, tag="v_ps")
                    nc.tensor.matmul(k_ps[:], lhsT=xnT[:], rhs=wsb["wk"][:], start=True, stop=True)
                    nc.tensor.matmul(v_ps[:], lhsT=xnT[:], rhs=wsb["wv"][:], start=True, stop=True)
                    if nontrivial_ln1:
                        nc.vector.tensor_tensor(
                            kvt[:, 0:C], k_ps[:],
                            bqkv_sb[:, C:2 * C].to_broadcast([128, C]), op=OP.add)
                        nc.vector.tensor_tensor(
                            kvt[:, C:2 * C], v_ps[:],
                            bqkv_sb[:, 2 * C:3 * C].to_broadcast([128, C]), op=OP.add)
                    else:
                        nc.scalar.copy(kvt[:, 0:C], k_ps[:])
                        nc.scalar.copy(kvt[:, C:2 * C], v_ps[:])
                    if ours:
                        q_ps = ps1.tile([128, 128], F32, tag="q_ps")
                        nc.tensor.matmul(q_ps[:], lhsT=xnT[:], rhs=wsb["wq"][:], start=True, stop=True)
                        if nontrivial_ln1:
                            nc.vector.tensor_tensor(
                                q_res[:, ts(t, 128)], q_ps[:],
                                bqkv_sb[:, 0:C].to_broadcast([128, C]), op=OP.add)
                        else:
                            nc.scalar.copy(q_res[:, ts(t, 128)], q_ps[:])
                    nc.sync.dma_start(kv_dram[ts(t, 128), :], kvt[:])

            # ---------------- Phase 2: gather + attention + MLP ----------------
            with (
                tc.tile_pool(name="p2", bufs=3) as p2,
                tc.tile_pool(name="p2s", bufs=2) as p2s,
                tc.tile_pool(name="ps2", bufs=1, space="PSUM") as ps2,
            ):
                for t in range(NT_HALF):
                    kvg = p2.tile([128, NJ * 2 * C], BF16, tag="kvg")
                    for j in range(NJ):
                        nc.gpsimd.indirect_dma_start(
                            out=kvg[:, ts(j, 2 * C)],
                            out_offset=None,
                            in_=kv_dram[:],
                            in_offset=bass.IndirectOffsetOnAxis(
                                ap=idx_res[:, t * NJ + j:t * NJ + j + 1], axis=0),
                        )
                    kvg_j = kvg[:].rearrange("p (j x) -> p j x", j=NJ)

                    # qk = sum_d q*kg per (j, head)
                    prod = p2s.tile([128, NJ * C], BF16, tag="prod")
                    qk = p2s.tile([128, NJ * NH], F32, tag="qk")
                    nc.vector.tensor_tensor(
                        prod[:].rearrange("p (j c) -> p j c", j=NJ),
                        q_res[:, ts(t, 128)].unsqueeze(1).to_broadcast([128, NJ, C]),
                        kvg_j[:, :, 0:C],
                        op=OP.mult,
                    )
                    tr8 = p2s.tile([128, NJ * NH * 8], BF16, tag="tr8")
                    tr4 = p2s.tile([128, NJ * NH * 4], BF16, tag="tr4")
                    tr2 = p2s.tile([128, NJ * NH * 2], BF16, tag="tr2")
                    p4d = prod[:].rearrange("p (j h d) -> p j h d", j=NJ, h=NH)
                    t8 = tr8[:].rearrange("p (j h d) -> p j h d", j=NJ, h=NH)
                    t4 = tr4[:].rearrange("p (j h d) -> p j h d", j=NJ, h=NH)
                    t2 = tr2[:].rearrange("p (j h d) -> p j h d", j=NJ, h=NH)
                    nc.vector.tensor_tensor(t8, p4d[:, :, :, 0:8], p4d[:, :, :, 8:16], op=OP.add)
                    nc.vector.tensor_tensor(t4, t8[:, :, :, 0:4], t8[:, :, :, 4:8], op=OP.add)
                    nc.vector.tensor_tensor(t2, t4[:, :, :, 0:2], t4[:, :, :, 2:4], op=OP.add)
                    nc.vector.tensor_tensor(
                        qk[:].rearrange("p (j h) -> p j h", j=NJ, h=NH).unsqueeze(3),
                        t2[:, :, :, 0:1], t2[:, :, :, 1:2], op=OP.add)
                    # softmax over j (no max subtraction; |qk| <~ 6)
                    E = p2s.tile([128, NJ * NH], BF16, tag="E")
                    sE = p2s.tile([128, NH], F32, tag="sE")
                    rec = p2s.tile([128, NH], F32, tag="rec")
                    A = p2s.tile([128, NJ * NH], BF16, tag="A")
                    nc.scalar.activation(E[:], qk[:], AF.Exp)
                    nc.vector.tensor_reduce(
                        sE[:], E[:].rearrange("p (j h) -> p h j", j=NJ), axis=AX.X, op=OP.add
                    )
                    nc.vector.reciprocal(rec[:], sE[:])
                    nc.vector.tensor_tensor(
                        A[:].rearrange("p (j h) -> p j h", j=NJ),
                        E[:].rearrange("p (j h) -> p j h", j=NJ),
                        rec[:].unsqueeze(1).to_broadcast([128, NJ, NH]),
                        op=OP.mult,
                    )
                    # att = sum_j A * vg
                    prod2 = p2s.tile([128, NJ * C], BF16, tag="prod2")
                    att = p2s.tile([128, C], BF16, tag="att")
                    Aexp = p2s.tile([128, NJ * C], BF16, tag="Aexp")
                    nc.scalar.activation(
                        Aexp[:].rearrange("p (j h d) -> p j h d", j=NJ, h=NH),
                        A[:].rearrange("p (j h) -> p j h", j=NJ).unsqueeze(3).to_broadcast([128, NJ, NH, HD]),
                        AF.Copy)
                    nc.vector.tensor_tensor(
                        prod2[:].rearrange("p (j c) -> p j c", j=NJ),
                        kvg_j[:, :, C:2 * C],
                        Aexp[:].rearrange("p (j c) -> p j c", j=NJ),
                        op=OP.mult,
                    )
                    av8 = p2s.tile([128, 8 * C], BF16, tag="av8")
                    av4 = p2s.tile([128, 4 * C], BF16, tag="av4")
                    av2 = p2s.tile([128, 2 * C], BF16, tag="av2")
                    nc.vector.tensor_tensor(av8[:], prod2[:, 0:8 * C], prod2[:, 8 * C:16 * C], op=OP.add)
                    nc.vector.tensor_tensor(av4[:], av8[:, 0:4 * C], av8[:, 4 * C:8 * C], op=OP.add)
                    nc.vector.tensor_tensor(av2[:], av4[:, 0:2 * C], av4[:, 2 * C:4 * C], op=OP.add)
                    nc.vector.tensor_tensor(att[:], av2[:, 0:C], av2[:, C:2 * C], op=OP.add)

                    # merge: qv = att @ Wm.T ; message = x0 + qv
                    attT_ps = ps2.tile([128, 128], BF16, tag="attT_ps")
                    attT = p2s.tile([128, 128], BF16, tag="attT")
                    nc.tensor.transpose(attT_ps[:], att[:], ident[:])
                    nc.scalar.copy(attT[:], attT_ps[:])
                    qv_ps = ps2.tile([128, 128], F32, tag="qv_ps")
                    nc.tensor.matmul(qv_ps[:], lhsT=attT[:], rhs=wsb["wm"][:], start=True, stop=True)
                    msg = p2s.tile([128, 128], BF16, tag="msg")
                    nc.vector.tensor_tensor(msg[:], x0_res[:, ts(t, 128)], qv_ps[:], op=OP.add)

                    # mlp
                    msgT_ps = ps2.tile([128, 128], BF16, tag="msgT_ps")
                    msgT = p2s.tile([128, 128], BF16, tag="msgT")
                    nc.tensor.transpose(msgT_ps[:], msg[:], ident[:])
                    nc.scalar.copy(msgT[:], msgT_ps[:])
                    m1_ps = ps2.tile([128, 128], F32, tag="m1_ps")
                    nc.tensor.matmul(m1_ps[:], lhsT=msgT[:], rhs=wsb["w1"][:], start=True, stop=True)
                    m1 = p2s.tile([128, 128], BF16, tag="m1")
                    nc.scalar.activation(m1[:], m1_ps[:], AF.Relu)
                    m1T_ps = ps2.tile([128, 128], BF16, tag="m1T_ps")
                    m1T = p2s.tile([128, 128], BF16, tag="m1T")
                    nc.tensor.transpose(m1T_ps[:], m1[:], ident[:])
                    nc.scalar.copy(m1T[:], m1T_ps[:])
                    m2_ps = ps2.tile([128, 128], F32, tag="m2_ps")
                    nc.tensor.matmul(m2_ps[:], lhsT=m1T[:], rhs=wsb["w2"][:], start=True, stop=True)

                    # LN2 + residual
                    m2sb = p2s.tile([128, 128], F32, tag="m2sb")
                    nc.scalar.activation(m2sb[:], m2_ps[:], AF.Copy)
                    ssum = p2s.tile([128, 1], F32, tag="ssum2")
                    mu = p2s.tile([128, 1], F32, tag="mu2")
                    sq = p2s.tile([128, 128], F32, tag="sq2")
                    m2st = p2s.tile([128, 1], F32, tag="m2st")
                    var = p2s.tile([128, 1], F32, tag="var2")
                    std = p2s.tile([128, 1], F32, tag="std2")
                    rstd = p2s.tile([128, 1], F32, tag="rstd2")
                    nc.scalar.activation(sq[:], m2sb[:], AF.Square, accum_out=m2st[:])
                    nc.scalar.activation(sqc2 := p2s.tile([128, 128], BF16, tag="sqc2", name="sqc2"),
                                         m2sb[:], AF.Copy, accum_out=ssum[:])
                    nc.vector.tensor_scalar_mul(mu[:], ssum[:], 1.0 / C)
                    nc.vector.tensor_scalar(
                        var[:], mu[:], scalar1=mu[:], scalar2=None, op0=OP.mult
                    )
                    nc.vector.tensor_scalar(
                        m2s2b := p2s.tile([128, 1], F32, tag="m2s2b", name="m2s2b"),
                        m2st[:], scalar1=1.0 / C, scalar2=EPS, op0=OP.mult, op1=OP.add,
                    )
                    nc.vector.tensor_tensor(var[:], m2s2b, var[:], op=OP.subtract)
                    nc.scalar.activation(std[:], var[:], AF.Sqrt)
                    nc.vector.reciprocal(rstd[:], std[:])
                    zn = p2s.tile([128, 128], F32, tag="zn")
                    outt = p2s.tile([128, 128], F32, tag="outt")
                    nc.vector.tensor_scalar(
                        zn[:], m2sb[:], scalar1=mu[:], scalar2=rstd[:],
                        op0=OP.subtract, op1=OP.mult,
                    )
                    if nontrivial_ln2:
                        nc.vector.tensor_tensor(
                            zn[:], zn[:], g2b2_sb[:, 0:C].to_broadcast([128, C]), op=OP.mult)
                        nc.vector.tensor_tensor(
                            zn[:], zn[:], g2b2_sb[:, C:2 * C].to_broadcast([128, C]), op=OP.add)
                    nc.vector.tensor_tensor(outt[:], zn[:], x0_res[:, ts(t, 128)], op=OP.add)
                    nc.sync.dma_start(out[ts(t, 128), :], outt[:])

    nc.finalize()
    return nc


def prep_core_inputs(x0_img: np.ndarray, query_img: np.ndarray, half: int, w):
    """Host-side prep for one core. w: dict of raw f32 weights g1,b1,g2,b2,Wq..W2."""
    ofs = half * LH
    x0r = np.ascontiguousarray(np.roll(x0_img, -ofs, axis=0))
    lq = query_img[ofs:ofs + LH, :].astype(np.int64)
    lq = (lq - ofs) % L  # remap into rotated coordinates

    gidx = np.zeros((128, NT_HALF * NJ), np.int32)
    for t in range(NT_HALF):
        gidx[:, t * NJ:(t + 1) * NJ] = lq[t * 128:(t + 1) * 128, :]
    bf = ml_dtypes.bfloat16
    g1 = w["g1"]
    m = {
        "x0f": x0r,
        "gidx": gidx,
        "ident": np.eye(C, dtype=np.float32).astype(bf),
        "wq": np.ascontiguousarray((0.25 * w["Wq"] * g1[None, :]).T).astype(bf),
        "wk": np.ascontiguousarray((w["Wk"] * g1[None, :]).T).astype(bf),
        "wv": np.ascontiguousarray((w["Wv"] * g1[None, :]).T).astype(bf),
        "wm": np.ascontiguousarray(w["Wm"].T).astype(bf),
        "w1": np.ascontiguousarray(w["W1"].T).astype(bf),
        "w2": np.ascontiguousarray(w["W2"].T).astype(bf),
    }
    b1 = w["b1"]
    nontrivial_ln1 = bool(np.any(b1 != 0.0))
    if nontrivial_ln1:
        m["bqkv"] = np.concatenate(
            [0.25 * (w["Wq"] @ b1), w["Wk"] @ b1, w["Wv"] @ b1]
        ).reshape(1, 3 * C).astype(np.float32)
    nontrivial_ln2 = bool(np.any(w["g2"] != 1.0) or np.any(w["b2"] != 0.0))
    if nontrivial_ln2:
        m["g2b2"] = np.concatenate([w["g2"], w["b2"]]).reshape(1, 2 * C).astype(np.float32)
    return m, nontrivial_ln1, nontrivial_ln2


def kernel(**inputs):
    from concourse.bass_utils import run_bass_kernel_spmd

    x0 = np.asarray(inputs["x0"], np.float32)
    query = np.asarray(inputs["query"])
    w = {k: np.asarray(inputs[k], np.float32)
         for k in ["Wq", "Wk", "Wv", "Wm", "W1", "W2", "g1", "b1", "g2", "b2"]}
    B = x0.shape[0]

    in_maps = []
    nt1 = nt2 = False
    for c in range(8):
        b, half = c // 2, c % 2
        m, nt1, nt2 = prep_core_inputs(x0[b], np.asarray(query[b]), half, w)
        in_maps.append(m)

    nc = build_nc(nt1, nt2)
    res = run_bass_kernel_spmd(nc, in_maps, core_ids=list(range(8)))

    outp = np.empty((B, L, C), np.float32)
    for c in range(8):
        b, half = c // 2, c % 2
        outp[b, half * LH:(half + 1) * LH, :] = res.results[c]["out"]
    return outp



# revision 26
# speedup vs baseline: 1.0193x; 1.0193x over previous
"""Self-contained Trainium2 Bass kernel for sparse attention.

Sharding: 8 cores = (image b, L-half). Each core receives its image's x0
ROTATED so its own 4096 rows come first (gather indices are remapped on
the host to match). The core computes LN+K/V for all 8192 rows, writes
packed bf16 [k|v] rows to DRAM scratch, then per 256-row iter gathers
4096 neighbor rows with ONE batched indirect DMA (multi-column offset
table -> 1 SWDGE instruction instead of 32) and runs attention + merge
+ MLP + LN2 on-chip. Elementwise work is split across DVE / Pool /
Activation engines; the MLP chain runs in transposed space to avoid
extra PE transposes. No collectives.
"""
import numpy as np
import ml_dtypes

import concourse.bass as bass
import concourse.tile as tile
from concourse import bacc, mybir
from concourse.library_config import mlp as mlp_lib

F32 = mybir.dt.float32
BF16 = mybir.dt.bfloat16
I32 = mybir.dt.int32
I16 = mybir.dt.int16
AX = mybir.AxisListType
OP = mybir.AluOpType
AF = mybir.ActivationFunctionType
ts = bass.ts

L, C, NJ, NH, HD = 8192, 128, 16, 8, 16
LH = L // 2            # rows computed per core
NT = LH // 128         # 32 attention tiles
ND = L // 256          # 32 phase-1 double-tile iters (64 tiles)
NE = NT // 2           # 16 phase-2 double-tile iters
EPS = 1e-5
RC = 1.0 / C


def build_nc(nontrivial_ln1: bool, nontrivial_ln2: bool):
    nc = bacc.Bacc(None, target_bir_lowering=False, debug=False)

    x0f = nc.declare_dram_parameter("x0f", [L, C], F32, isOutput=False)
    gidx = nc.declare_dram_parameter("gidx", [128, NE * 256], I16, isOutput=False)
    wkvp = nc.declare_dram_parameter("wkv", [C, 2 * C], BF16, isOutput=False)
    wnames = ["wq", "wm", "w1", "w2"]
    wparams = {n: nc.declare_dram_parameter(n, [C, C], BF16, isOutput=False) for n in wnames}
    identp = nc.declare_dram_parameter("ident", [C, C], BF16, isOutput=False)
    identfp = nc.declare_dram_parameter("identf", [C, C], F32, isOutput=False)
    if nontrivial_ln1:
        bqkv = nc.declare_dram_parameter("bqkv", [1, 3 * C], F32, isOutput=False)
    if nontrivial_ln2:
        g2b2 = nc.declare_dram_parameter("g2b2", [1, 2 * C], F32, isOutput=False)
    out = nc.declare_dram_parameter("out", [LH, C], F32, isOutput=True)

    with tile.TileContext(nc) as tc:
        with (
            tc.tile_pool(name="res", bufs=1) as res,
            tc.tile_pool(name="dram", bufs=1, space="DRAM") as dram,
        ):
            kv_dram = dram.tile([L, 2 * C], BF16)
            x0_res = res.tile([128, NT * 128], F32)    # our rows, row-major (LN2 residual)
            x0T_res = res.tile([128, NT * 128], F32)   # our rows, transposed (merge residual)
            q_res = res.tile([128, NT * 128], BF16)    # our q tiles
            idx_res = res.tile([128, NE * 256], I16)
            ident = res.tile([128, 128], BF16)
            identf = res.tile([128, 128], F32)
            wkv_sb = res.tile([C, 2 * C], BF16)
            wsb = {n: res.tile([C, C], BF16, name=f"w_{n}", tag=f"w_{n}") for n in wnames}

            nc.sync.dma_start(idx_res[:], gidx[:])
            nc.sync.dma_start(wkv_sb[:], wkvp[:])
            for n in wnames:
                nc.sync.dma_start(wsb[n][:], wparams[n][:])
            nc.sync.dma_start(ident[:], identp[:])
            nc.sync.dma_start(identf[:], identfp[:])
            if nontrivial_ln1:
                bqkv_sb = res.tile([1, 3 * C], F32)
                nc.sync.dma_start(bqkv_sb[:], bqkv[:])
            if nontrivial_ln2:
                g2b2_sb = res.tile([1, 2 * C], F32)
                nc.sync.dma_start(g2b2_sb[:], g2b2[:])

            # ---------------- Phase 1: LN1 + K/V (+Q) projections ----------------
            # 256 rows (2 tiles) per iter; LN1 stats on DVE, apply on DVE,
            # transposes on PE, PSUM->SBUF copies split scalar/pool, kv
            # written to DRAM directly from PSUM.
            with (
                tc.tile_pool(name="p1", bufs=3) as p1,
                tc.tile_pool(name="p1s", bufs=2) as p1s,
                tc.tile_pool(name="ps1", bufs=2, space="PSUM") as ps1,
            ):
                for d in range(ND):
                    ours = d < NT // 2
                    if ours:
                        x0d = x0_res[:, ts(d, 256)]
                    else:
                        x0d_t = p1.tile([128, 256], F32, tag="x0d")
                        x0d = x0d_t[:]
                    # rows d*256 .. d*256+255 -> [u, p, c]
                    nc.sync.dma_start(
                        x0d.rearrange("p (u c) -> p u c", u=2),
                        x0f[ts(d, 256), :].rearrange("(u p) c -> p u c", u=2),
                    )
                    x0v = x0d.rearrange("p (u c) -> p u c", u=2)

                    sq = p1s.tile([128, 256], F32, tag="sq")
                    ssq = p1s.tile([128, 2], F32, tag="ssq")
                    ssum = p1s.tile([128, 2], F32, tag="ssum")
                    mun = p1s.tile([128, 2], F32, tag="mun")
                    musq = p1s.tile([128, 2], F32, tag="musq")
                    var = p1s.tile([128, 2], F32, tag="var")
                    std = p1s.tile([128, 2], F32, tag="std")
                    rstd = p1s.tile([128, 2], F32, tag="rstd")
                    nc.vector.tensor_tensor(sq[:], x0d, x0d, op=OP.mult)
                    nc.vector.tensor_reduce(
                        ssq[:], sq[:].rearrange("p (u c) -> p u c", u=2),
                        axis=AX.X, op=OP.add)
                    nc.vector.tensor_reduce(ssum[:], x0v, axis=AX.X, op=OP.add)
                    nc.vector.tensor_scalar_mul(mun[:], ssum[:], -RC)
                    nc.vector.tensor_tensor(musq[:], mun[:], mun[:], op=OP.mult)
                    nc.vector.tensor_scalar(
                        var[:], ssq[:], scalar1=RC, scalar2=EPS,
                        op0=OP.mult, op1=OP.add)
                    nc.vector.tensor_tensor(var[:], var[:], musq[:], op=OP.subtract)
                    nc.scalar.activation(std[:], var[:], AF.Sqrt)
                    nc.vector.reciprocal(rstd[:], std[:])

                    xn = p1.tile([128, 256], BF16, tag="xn")
                    xnv = xn[:].rearrange("p (u c) -> p u c", u=2)
                    for u in range(2):
                        nc.vector.tensor_scalar(
                            xnv[:, u], x0v[:, u],
                            scalar1=mun[:, u:u + 1], scalar2=rstd[:, u:u + 1],
                            op0=OP.add, op1=OP.mult,
                        )

                    xnT_ps = ps1.tile([128, 256], BF16, tag="xnT_ps")
                    xnT = p1.tile([128, 256], BF16, tag="xnT")
                    for u in range(2):
                        nc.tensor.transpose(xnT_ps[:, ts(u, 128)], xnv[:, u], ident[:])
                    nc.scalar.copy(xnT[:], xnT_ps[:])

                    kv_ps = ps1.tile([128, 512], F32, tag="kv_ps")
                    for u in range(2):
                        nc.tensor.matmul(kv_ps[:, ts(u, 256)],
                                         lhsT=xnT[:, ts(u, 128)], rhs=wkv_sb[:],
                                         start=True, stop=True)
                    kvt = p1.tile([128, 512], BF16, tag="kvt")
                    if nontrivial_ln1:
                        kvv = kv_ps[:].rearrange("p (u kv c) -> p u kv c", u=2, kv=2)
                        ktv = kvt[:].rearrange("p (u kv c) -> p u kv c", u=2, kv=2)
                        bb = bqkv_sb[:, C:3 * C].rearrange("o (kv c) -> o kv c", kv=2)
                        nc.vector.tensor_tensor(
                            ktv, kvv, bb.unsqueeze(1).to_broadcast([128, 2, 2, C]),
                            op=OP.add)
                    else:
                        nc.scalar.copy(kvt[:], kv_ps[:])
                    nc.sync.dma_start(
                        kv_dram[ts(d, 256), :].rearrange("(u p) c -> p u c", u=2),
                        kvt[:].rearrange("p (u c) -> p u c", u=2),
                    )

                    if ours:
                        q_ps = ps1.tile([128, 256], F32, tag="q_ps")
                        for u in range(2):
                            nc.tensor.matmul(q_ps[:, ts(u, 128)],
                                             lhsT=xnT[:, ts(u, 128)], rhs=wsb["wq"][:],
                                             start=True, stop=True)
                        if nontrivial_ln1:
                            nc.vector.tensor_tensor(
                                q_res[:, ts(d, 256)].rearrange("p (u c) -> p u c", u=2),
                                q_ps[:].rearrange("p (u c) -> p u c", u=2),
                                bqkv_sb[:, 0:C].unsqueeze(1).to_broadcast([128, 2, C]),
                                op=OP.add)
                        else:
                            nc.scalar.copy(q_res[:, ts(d, 256)], q_ps[:])

                        x0T_ps = ps1.tile([128, 256], F32, tag="x0T_ps")
                        for u in range(2):
                            nc.tensor.transpose(x0T_ps[:, ts(u, 128)], x0v[:, u], identf[:])
                        nc.scalar.copy(x0T_res[:, ts(d, 256)], x0T_ps[:])

            # ---------------- Phase 2: gather + attention + MLP ----------------
            nc.gpsimd.load_library(mlp_lib)
            with (
                tc.tile_pool(name="p2", bufs=3) as p2,
                tc.tile_pool(name="p2s", bufs=2) as p2s,
                tc.tile_pool(name="ps2", bufs=2, space="PSUM") as ps2,
            ):
                for e in range(NE):
                    kvg = p2.tile([128, 32 * 256], BF16, tag="kvg", bufs=2)
                    kvgv = kvg[:].rearrange("p (c x) -> p c x", c=32)
                    # dma_gather wedges the device above ~1024 idxs/instr;
                    # split each 2-tile gather into 4x1024.
                    for g in range(4):
                        nc.gpsimd.dma_gather(
                            kvgv[:, g * 8:(g + 1) * 8, :],
                            kv_dram[:],
                            idx_res[:, e * 256 + g * 64:e * 256 + (g + 1) * 64],
                            1024, 1024, 256)
                    kv4 = kvg[:].rearrange("p (t j x) -> p t j x", t=2, j=NJ)
                    kg = kv4[:, :, :, 0:C]                      # (h d) order
                    vg = kv4[:, :, :, C:2 * C].rearrange(
                        "p t j (dd h) -> p t j dd h", h=NH)     # (d h) order

                    # ---- scores: qk[p,t,j,h] = sum_d q*k ----
                    prod = p2s.tile([128, 2 * NJ * C], BF16, tag="prod")
                    pv = prod[:].rearrange("p (t j c) -> p t j c", t=2, j=NJ)
                    qb = q_res[:, ts(e, 256)].rearrange("p (t c) -> p t c", t=2)
                    nc.vector.tensor_tensor(
                        pv, qb.unsqueeze(2).to_broadcast([128, 2, NJ, C]), kg,
                        op=OP.mult)
                    p5 = prod[:].rearrange("p (t j h dd) -> p t j h dd", t=2, j=NJ, h=NH)
                    t8 = p2s.tile([128, 2 * NJ * NH * 8], BF16, tag="t8")
                    t4 = p2s.tile([128, 2 * NJ * NH * 4], BF16, tag="t4")
                    t2 = p2s.tile([128, 2 * NJ * NH * 2], BF16, tag="t2")
                    qk = p2s.tile([128, 2 * NJ * NH], BF16, tag="qk")
                    t8v = t8[:].rearrange("p (t j h dd) -> p t j h dd", t=2, j=NJ, h=NH)
                    t4v = t4[:].rearrange("p (t j h dd) -> p t j h dd", t=2, j=NJ, h=NH)
                    t2v = t2[:].rearrange("p (t j h dd) -> p t j h dd", t=2, j=NJ, h=NH)
                    nc.vector.tensor_tensor(t8v, p5[:, :, :, :, 0:8], p5[:, :, :, :, 8:16], op=OP.add)
                    nc.vector.tensor_tensor(t4v, t8v[:, :, :, :, 0:4], t8v[:, :, :, :, 4:8], op=OP.add)
                    nc.vector.tensor_tensor(t2v, t4v[:, :, :, :, 0:2], t4v[:, :, :, :, 2:4], op=OP.add)
                    nc.vector.tensor_tensor(
                        qk[:].rearrange("p (t j h) -> p t j h", t=2, j=NJ).unsqueeze(4),
                        t2v[:, :, :, :, 0:1], t2v[:, :, :, :, 1:2], op=OP.add)

                    # ---- softmax pieces: E = exp(qk); sE = sum_j E; rec = 1/sE ----
                    E = p2s.tile([128, 2 * NJ * NH], BF16, tag="E")
                    e8 = p2s.tile([128, 2 * 8 * NH], BF16, tag="e8")
                    e4 = p2s.tile([128, 2 * 4 * NH], BF16, tag="e4")
                    e2 = p2s.tile([128, 2 * 2 * NH], BF16, tag="e2")
                    sE = p2s.tile([128, 2 * NH], F32, tag="sE")
                    rec = p2s.tile([128, 2 * NH], BF16, tag="rec")
                    nc.scalar.activation(E[:], qk[:], AF.Exp)
                    Ev = E[:].rearrange("p (t j h) -> p t j h", t=2, j=NJ)
                    e8v = e8[:].rearrange("p (t j h) -> p t j h", t=2, j=8)
                    e4v = e4[:].rearrange("p (t j h) -> p t j h", t=2, j=4)
                    e2v = e2[:].rearrange("p (t j h) -> p t j h", t=2, j=2)
                    nc.vector.tensor_tensor(e8v, Ev[:, :, 0:8], Ev[:, :, 8:16], op=OP.add)
                    nc.vector.tensor_tensor(e4v, e8v[:, :, 0:4], e8v[:, :, 4:8], op=OP.add)
                    nc.vector.tensor_tensor(e2v, e4v[:, :, 0:2], e4v[:, :, 2:4], op=OP.add)
                    nc.vector.tensor_tensor(
                        sE[:].rearrange("p (t h) -> p t h", t=2).unsqueeze(2),
                        e2v[:, :, 0:1], e2v[:, :, 1:2], op=OP.add)
                    with nc.allow_low_precision(reason="softmax denom in bf16 is fine at 2e-2 tol"):
                        nc.vector.reciprocal(rec[:], sE[:])

                    # ---- att_raw[p,t,(d h)] = sum_j E * vg ----
                    prod2 = p2s.tile([128, 2 * NJ * C], BF16, tag="prod2")
                    p2v = prod2[:].rearrange("p (t j dd h) -> p t j dd h", t=2, j=NJ, h=NH)
                    Eb = E[:].rearrange("p (t j h) -> p t j h", t=2, j=NJ).unsqueeze(3)
                    nc.vector.tensor_tensor(
                        p2v, Eb.to_broadcast([128, 2, NJ, HD, NH]), vg, op=OP.mult)
                    p2j = prod2[:].rearrange("p (t j c) -> p t j c", t=2, j=NJ)
                    av8 = p2s.tile([128, 2 * 8 * C], BF16, tag="av8")
                    av4 = p2s.tile([128, 2 * 4 * C], BF16, tag="av4")
                    av2 = p2s.tile([128, 2 * 2 * C], BF16, tag="av2")
                    atr = p2s.tile([128, 2 * C], BF16, tag="atr")
                    av8v = av8[:].rearrange("p (t j c) -> p t j c", t=2, j=8)
                    av4v = av4[:].rearrange("p (t j c) -> p t j c", t=2, j=4)
                    av2v = av2[:].rearrange("p (t j c) -> p t j c", t=2, j=2)
                    nc.vector.tensor_tensor(av8v, p2j[:, :, 0:8], p2j[:, :, 8:16], op=OP.add)
                    nc.vector.tensor_tensor(av4v, av8v[:, :, 0:4], av8v[:, :, 4:8], op=OP.add)
                    nc.vector.tensor_tensor(av2v, av4v[:, :, 0:2], av4v[:, :, 2:4], op=OP.add)
                    nc.vector.tensor_tensor(
                        atr[:].rearrange("p (t c) -> p t c", t=2).unsqueeze(2),
                        av2v[:, :, 0:1], av2v[:, :, 1:2], op=OP.add)

                    # normalize: att = att_raw * (1/sE) broadcast over d
                    attn = p2s.tile([128, 2 * C], BF16, tag="attn")
                    nc.vector.tensor_tensor(
                        attn[:].rearrange("p (t dd h) -> p t dd h", t=2, h=NH),
                        atr[:].rearrange("p (t dd h) -> p t dd h", t=2, h=NH),
                        rec[:].rearrange("p (t h) -> p t h", t=2)
                        .unsqueeze(2).to_broadcast([128, 2, HD, NH]),
                        op=OP.mult)

                    # ---- merge + MLP in transposed space ----
                    attT_ps = ps2.tile([128, 256], BF16, tag="attT_ps")
                    attT = p2s.tile([128, 256], BF16, tag="attT")
                    for u in range(2):
                        nc.tensor.transpose(attT_ps[:, ts(u, 128)],
                                            attn[:, ts(u, 128)], ident[:])
                    nc.scalar.copy(attT[:], attT_ps[:])

                    qvT_ps = ps2.tile([128, 256], F32, tag="qvT_ps")
                    for u in range(2):
                        nc.tensor.matmul(qvT_ps[:, ts(u, 128)], lhsT=wsb["wm"][:],
                                         rhs=attT[:, ts(u, 128)], start=True, stop=True)
                    msgT = p2s.tile([128, 256], BF16, tag="msgT")
                    nc.vector.tensor_tensor(
                        msgT[:], x0T_res[:, ts(e, 256)], qvT_ps[:], op=OP.add)

                    m1T_ps = ps2.tile([128, 256], F32, tag="m1T_ps")
                    m1T = p2s.tile([128, 256], BF16, tag="m1T")
                    for u in range(2):
                        nc.tensor.matmul(m1T_ps[:, ts(u, 128)], lhsT=wsb["w1"][:],
                                         rhs=msgT[:, ts(u, 128)], start=True, stop=True)
                    nc.scalar.activation(m1T[:], m1T_ps[:], AF.Relu)

                    m2_ps = ps2.tile([128, 256], F32, tag="m2_ps")
                    for u in range(2):
                        nc.tensor.matmul(m2_ps[:, ts(u, 128)], lhsT=m1T[:, ts(u, 128)],
                                         rhs=wsb["w2"][:], start=True, stop=True)
                    m2v = m2_ps[:].rearrange("p (u c) -> p u c", u=2)

                    # ---- LN2 (row space) + residual ----
                    sq2 = p2s.tile([128, 256], F32, tag="sq2")
                    cp2 = p2s.tile([128, 256], F32, tag="cp2")
                    ssq2 = p2s.tile([128, 2], F32, tag="ssq2")
                    ssum2 = p2s.tile([128, 2], F32, tag="ssum2")
                    mun2 = p2s.tile([128, 2], F32, tag="mun2")
                    musq2 = p2s.tile([128, 2], F32, tag="musq2")
                    var2 = p2s.tile([128, 2], F32, tag="var2")
                    std2 = p2s.tile([128, 2], F32, tag="std2")
                    rstd2 = p2s.tile([128, 2], F32, tag="rstd2")
                    for u in range(2):
                        nc.scalar.activation(cp2[:, ts(u, 128)], m2v[:, u], AF.Copy,
                                             accum_out=ssum2[:, u:u + 1])
                    nc.vector.tensor_tensor(sq2[:], cp2[:], cp2[:], op=OP.mult)
                    nc.vector.tensor_reduce(
                        ssq2[:], sq2[:].rearrange("p (u c) -> p u c", u=2),
                        axis=AX.X, op=OP.add)
                    nc.vector.tensor_scalar_mul(mun2[:], ssum2[:], -RC)
                    nc.vector.tensor_tensor(musq2[:], mun2[:], mun2[:], op=OP.mult)
                    nc.vector.tensor_scalar(
                        var2[:], ssq2[:], scalar1=RC, scalar2=EPS,
                        op0=OP.mult, op1=OP.add)
                    nc.vector.tensor_tensor(var2[:], var2[:], musq2[:], op=OP.subtract)
                    nc.scalar.activation(std2[:], var2[:], AF.Sqrt)
                    nc.vector.reciprocal(rstd2[:], std2[:])

                    zn = p2s.tile([128, 256], F32, tag="zn")
                    znv = zn[:].rearrange("p (u c) -> p u c", u=2)
                    for u in range(2):
                        nc.vector.tensor_scalar(
                            znv[:, u], m2v[:, u],
                            scalar1=mun2[:, u:u + 1], scalar2=rstd2[:, u:u + 1],
                            op0=OP.add, op1=OP.mult,
                        )
                    if nontrivial_ln2:
                        nc.vector.tensor_tensor(
                            znv, znv,
                            g2b2_sb[:, 0:C].unsqueeze(1).to_broadcast([128, 2, C]),
                            op=OP.mult)
                        nc.vector.tensor_tensor(
                            znv, znv,
                            g2b2_sb[:, C:2 * C].unsqueeze(1).to_broadcast([128, 2, C]),
                            op=OP.add)
                    outt = p2s.tile([128, 256], F32, tag="outt")
                    nc.vector.tensor_tensor(outt[:], zn[:], x0_res[:, ts(e, 256)], op=OP.add)
                    nc.sync.dma_start(
                        out[ts(e, 256), :].rearrange("(u p) c -> p u c", u=2),
                        outt[:].rearrange("p (u c) -> p u c", u=2),
                    )

    nc.finalize()
    return nc


def prep_core_inputs(x0_img: np.ndarray, query_img: np.ndarray, half: int, w):
    """Host-side prep for one core. w: dict of raw f32 weights g1,b1,g2,b2,Wq..W2."""
    ofs = half * LH
    x0r = np.ascontiguousarray(np.roll(x0_img, -ofs, axis=0))
    lq = query_img[ofs:ofs + LH, :].astype(np.int64)
    lq = (lq - ofs) % L  # remap into rotated coordinates

    # dma_gather idx table: per phase-2 iter e a [128, 256] int16 block;
    # gathered ordinal i = c*128+p lives at [i % 16, i // 16], replicated
    # to all 128 partitions. value = lq[(2e + c//16)*128 + p, c % 16].
    gidx = np.zeros((128, NE * 256), np.int16)
    lq3 = lq.reshape(NT, 128, NJ)
    for e in range(NE):
        blk = lq3[2 * e:2 * e + 2]                      # [2, 128, NJ]
        arr = blk.transpose(0, 2, 1).reshape(32, 128)   # [c=(t j), p]
        for g in range(4):
            sub = arr[g * 8:(g + 1) * 8].reshape(1024)  # ordinals of block g
            tbl = sub.reshape(64, 16).T                 # [16, 64]
            gidx[:, e * 256 + g * 64:e * 256 + (g + 1) * 64] = np.tile(tbl, (8, 1))
    bf = ml_dtypes.bfloat16
    g1 = w["g1"]
    # v channels permuted to (d-major, h-minor); Wm rows permuted to match.
    perm = np.array([(cp % NH) * HD + cp // NH for cp in range(C)], np.int64)
    wk_t = (w["Wk"] * g1[None, :]).T                         # [c, c2]
    wv_t = (w["Wv"] * g1[None, :]).T[:, perm]
    m = {
        "x0f": x0r,
        "gidx": gidx,
        "ident": np.eye(C, dtype=np.float32).astype(bf),
        "identf": np.eye(C, dtype=np.float32),
        "wkv": np.ascontiguousarray(np.concatenate([wk_t, wv_t], axis=1)).astype(bf),
        "wq": np.ascontiguousarray((0.25 * w["Wq"] * g1[None, :]).T).astype(bf),
        "wm": np.ascontiguousarray(w["Wm"].T[perm, :]).astype(bf),
        "w1": np.ascontiguousarray(w["W1"].T).astype(bf),
        "w2": np.ascontiguousarray(w["W2"].T).astype(bf),
    }
    b1 = w["b1"]
    nontrivial_ln1 = bool(np.any(b1 != 0.0))
    if nontrivial_ln1:
        qb = 0.25 * (w["Wq"] @ b1)
        kb = w["Wk"] @ b1
        vb = (w["Wv"] @ b1)[perm]
        m["bqkv"] = np.concatenate([qb, kb, vb]).reshape(1, 3 * C).astype(np.float32)
    nontrivial_ln2 = bool(np.any(w["g2"] != 1.0) or np.any(w["b2"] != 0.0))
    if nontrivial_ln2:
        m["g2b2"] = np.concatenate([w["g2"], w["b2"]]).reshape(1, 2 * C).astype(np.float32)
    return m, nontrivial_ln1, nontrivial_ln2


def kernel(**inputs):
    from concourse.bass_utils import run_bass_kernel_spmd

    x0 = np.asarray(inputs["x0"], np.float32)
    query = np.asarray(inputs["query"])
    w = {k: np.asarray(inputs[k], np.float32)
         for k in ["Wq", "Wk", "Wv", "Wm", "W1", "W2", "g1", "b1", "g2", "b2"]}
    B = x0.shape[0]

    in_maps = []
    nt1 = nt2 = False
    for c in range(8):
        b, half = c // 2, c % 2
        m, nt1, nt2 = prep_core_inputs(x0[b], np.asarray(query[b]), half, w)
        in_maps.append(m)

    nc = build_nc(nt1, nt2)
    res = run_bass_kernel_spmd(nc, in_maps, core_ids=list(range(8)))

    outp = np.empty((B, L, C), np.float32)
    for c in range(8):
        b, half = c // 2, c % 2
        outp[b, half * LH:(half + 1) * LH, :] = res.results[c]["out"]
    return outp
